# revision 8
# baseline (speedup 1.0000x reference)
"""Causal self-attention (GPT-style block) on 8 Trainium2 NeuronCores.

Problem: x[4,2048,1024] -> qkv = x@W_attn+b ; 16-head causal attention
(head_dim 64) ; out = y@W_proj+b_proj.

Sharding: tensor-parallel over heads. Core c owns heads {2c, 2c+1}:
  - computes q^T/k^T (feature-major) and v (key-major, natural layout)
    for its heads over the full batch via matmuls against a host-
    pretransposed x^T (bf16). v-natural comes from using the x^T tile as
    the stationary operand (out partitions = tokens), so no DMA
    transposes / DRAM round trip are needed.
  - runs causal attention for its 8 (batch, head) pairs entirely in SBUF
    (S^T layout: scores tile [128 j, 512 i]; j-tiles processed in pairs
    sharing one [128, 1024] PSUM tile so exp runs as one ScalarE
    activation per pair; causal mask via affine_select on the diagonal
    band only - fully masked leading columns are simply never read,
    because the PV matmul accumulates into yps[:, i0:] per tile; PV
    appends a ones-column to v producing y_raw^T and the softmax
    denominator in one PSUM tile),
  - per batch, a striped AllToAll (launched as soon as that batch's
    attention finishes, overlapping the next batch's compute) swaps the
    head dim for the row dim; the output projection for the core's
    4x256 rows runs after attention, pipelined with the last collective.

Bias handling: b_k drops out of softmax exactly (constant per query row);
b_v is folded into b_proj on the host (attention rows sum to 1); only
b_q is applied on device.

Numerics: bf16 operands with fp32 PSUM accumulation everywhere; softmax
skips the max-subtraction (scores are O(1) by construction; exp stays
finite) which matches the reference to ~1e-5 in fp32.
"""

import numpy as np
import ml_dtypes
from contextlib import ExitStack

import concourse.bass as bass
import concourse.tile as tile
from concourse import bacc, mybir
from concourse.bass_utils import run_bass_kernel_spmd

F32 = mybir.dt.float32
BF16 = mybir.dt.bfloat16
AF = mybir.ActivationFunctionType

N_CORES = 8
B, T, C, H = 4, 2048, 1024, 16
HD = C // H            # 64 head dim
HPC = H // N_CORES     # 2 heads per core
FPC = HPC * HD         # 128 features per core
BT = B * T             # 8192 rows
TCHUNK = 512           # t chunk in qkv phase
NT_CHUNKS = BT // TCHUNK
QB = 512               # query block
NQB = T // QB          # 4 per batch
JTN = T // 128         # 16 j-tiles per batch
ROWS = BT // N_CORES   # 1024 rows per core after AllToAll
KC = C // 128          # 8 contraction tiles over C
VW = 80                # [V | 1 | pad] row unit in vsb
STRIPE = ROWS // B     # 256 rows per (core, batch): striped AllToAll
SCALE = 1.0 / np.sqrt(HD)

# If the overlapped (per-batch, concurrent-with-compute) AllToAll turns
# out to corrupt data on HW, set False to emit all collectives after the
# attention loop (still chunked and pipelined with the out projection).
OVERLAP_A2A = True

# AllToAll chunking: chunk i covers CHUNK_L[i] 128-row blocks per core,
# i.e. blocks [8*B0[i], 8*B0[i+1]) globally. Sized so the last chunk is
# small (fires after the final qb and gates the tail).
CHUNK_L = (4, 2, 1, 1)
CHUNK_B0 = (0, 4, 6, 7)  # cumulative blocks-per-dest before each chunk

LAST_RESULTS = None    # test.py reads exec_time_ns off this


def build_program(nc):
    xT = nc.dram_tensor("xT", [C, BT], BF16, kind="ExternalInput").ap()
    wq = nc.dram_tensor("wq", [C, FPC], BF16, kind="ExternalInput").ap()
    wk = nc.dram_tensor("wk", [C, FPC], BF16, kind="ExternalInput").ap()
    wv = nc.dram_tensor("wv", [C, FPC], BF16, kind="ExternalInput").ap()
    bqkv = nc.dram_tensor("bqkv", [3, FPC], F32, kind="ExternalInput").ap()
    wp = nc.dram_tensor("wp", [C, C], BF16, kind="ExternalInput").ap()
    bp = nc.dram_tensor("bp", [C], F32, kind="ExternalInput").ap()
    out = nc.dram_tensor("out", [ROWS, C], F32, kind="ExternalOutput").ap()
    # round-robin 128-row blocks: global block m (of 64) -> core m%8, so a
    # collective chunk covering any 8*L consecutive blocks addresses all 8
    # destinations. Chunks fire after b1, b2, (b3,qb1), (b3,qb3) with
    # L = 4, 2, 1, 1 blocks per destination.
    cc_in = [
        nc.dram_tensor(f"cc_in{i}", [N_CORES, FPC, L * 128], BF16, kind="Internal").ap()
        for i, L in enumerate(CHUNK_L)
    ]
    cc_out = [
        nc.dram_tensor(f"cc_out{i}", [N_CORES, FPC, L * 128], BF16, kind="Internal").ap()
        for i, L in enumerate(CHUNK_L)
    ]

    with tile.TileContext(nc) as tc:
        with ExitStack() as ctx:
            emit(ctx, tc, xT, wq, wk, wv, bqkv, wp, bp, out, cc_in, cc_out)
    return nc


def emit(ctx, tc, xT, wq, wk, wv, bqkv, wp, bp, out, cc_in, cc_out):
    nc = tc.nc
    res = ctx.enter_context(tc.tile_pool(name="resident", bufs=1))

    # ---------- resident SBUF ----------
    qT = res.tile([128, BT], BF16)
    kT = res.tile([128, BT], BF16)
    vsb = res.tile([128, B * JTN, HPC, VW], BF16)     # [V | 1 | pad] per j-tile/head
    wq_sb = res.tile([128, KC, FPC], BF16)
    wk_sb = res.tile([128, KC, FPC], BF16)
    wv_sb = res.tile([128, KC, FPC], BF16)
    b_sb = res.tile([128, 3], F32)
    wp_sb = res.tile([128, KC, C], BF16)
    bp_sb = res.tile([128, C], F32)
    yT0 = res.tile([64, BT], BF16)
    yT1 = res.tile([64, BT], BF16)

    # ---------- constant/weight loads (wp/bp deferred to phase 3) ----------
    nc.sync.dma_start(wq_sb[:], wq.rearrange("(a p) m -> p a m", p=128))
    nc.sync.dma_start(wk_sb[:], wk.rearrange("(a p) m -> p a m", p=128))
    nc.sync.dma_start(wv_sb[:], wv.rearrange("(a p) m -> p a m", p=128))
    nc.sync.dma_start(b_sb[:], bqkv.rearrange("b p -> p b"))
    nc.vector.memset(vsb[:, :, :, HD : HD + 1], 1.0)

    # ---------- phase 1: q^T, k^T (feature-major) and v (natural) ----------
    ph12 = ExitStack()
    xpool = ph12.enter_context(tc.tile_pool(name="xt", bufs=3))
    qkvps = ph12.enter_context(tc.tile_pool(name="qkvps", bufs=2, space="PSUM"))
    vps = ph12.enter_context(tc.tile_pool(name="vps", bufs=2, space="PSUM"))
    xT_t = xT.rearrange("(a p) t -> p a t", p=128)
    NSUB = TCHUNK // 128
    for tci in range(NT_CHUNKS):
        t0 = tci * TCHUNK
        xt = xpool.tile([128, KC, TCHUNK], BF16, tag="xt")
        # split the 1 MiB chunk load across 4 DMA queues
        for spl in range(4):
            nc.sync.dma_start(
                xt[:, 2 * spl : 2 * spl + 2, :],
                xT_t[:, 2 * spl : 2 * spl + 2, t0 : t0 + TCHUNK],
            )
        for w_sb, bi, dst in ((wq_sb, 0, qT), (wk_sb, 1, kT)):
            ps = qkvps.tile([128, TCHUNK], F32, tag="qkvps")
            for a in range(KC):
                nc.tensor.matmul(
                    ps[:], lhsT=w_sb[:, a, :], rhs=xt[:, a, :],
                    start=(a == 0), stop=(a == KC - 1),
                )
            # evictions on DVE: keeps ScalarE exp-only (no ACT table switches)
            if bi == 0:
                nc.vector.tensor_scalar_add(
                    dst[:, t0 : t0 + TCHUNK], ps[:], b_sb[:, bi : bi + 1]
                )
            else:
                # b_k shifts every score in a query row equally -> softmax
                # invariant; drop it.
                nc.vector.tensor_copy(dst[:, t0 : t0 + TCHUNK], ps[:])
        # v in natural layout: out partitions = tokens (keys), free = feature.
        # lhsT = x^T tile (stationary), rhs = W_v tile (moving).
        vp = vps.tile([128, NSUB, HPC, HD], F32, tag="vps")
        for tt in range(NSUB):
            for a in range(KC):
                nc.tensor.matmul(
                    vp[:, tt, :, :],
                    lhsT=xt[:, a, tt * 128 : (tt + 1) * 128],
                    rhs=wv_sb[:, a, :],
                    start=(a == 0), stop=(a == KC - 1),
                )
        # b_v is folded into b_proj on the host (attention rows sum to 1)
        g0 = NSUB * tci
        nc.vector.tensor_copy(vsb[:, g0 : g0 + NSUB, :, 0:HD], vp[:])

    ph12.close()  # release phase-1 PSUM/xt pools before the attention pools open

    nc.sync.dma_start(wp_sb[:], wp.rearrange("(a p) e -> p a e", p=128))
    bp_bcast = bass.AP(tensor=bp.tensor, offset=bp.offset, ap=[[0, 128], [1, C]])
    nc.sync.dma_start(bp_sb[:], bp_bcast)

    # ---------- phase 2: causal attention, S^T layout, heads interleaved ----------
    # Interleaving the two heads keeps consecutive PE matmuls independent
    # (different array row groups for S, different PSUM banks throughout),
    # so LDWEIGHTS/fill/drain overlap instead of serializing.
    ph23 = ExitStack()
    spool = ph23.enter_context(tc.tile_pool(name="sps", bufs=3, space="PSUM"))
    ypool = ph23.enter_context(tc.tile_pool(name="yps", bufs=2, space="PSUM"))
    ptpool = ph23.enter_context(tc.tile_pool(name="pt", bufs=3))
    npool = ph23.enter_context(tc.tile_pool(name="norm", bufs=3))
    yT = (yT0, yT1)

    def stage_qb(b, qb):
        # stage this qb's four 128-row blocks into their chunk slots
        for u in range(4):
            m = 16 * b + 4 * qb + u
            i = next(i for i in range(4)
                     if 8 * CHUNK_B0[i] <= m < 8 * (CHUNK_B0[i] + CHUNK_L[i]))
            r, l = m % 8, (m - 8 * CHUNK_B0[i]) // 8
            c0 = m * 128
            nc.sync.dma_start(cc_in[i][r, 0:HD, l * 128 : (l + 1) * 128],
                              yT0[:, c0 : c0 + 128])
            nc.sync.dma_start(cc_in[i][r, HD:FPC, l * 128 : (l + 1) * 128],
                              yT1[:, c0 : c0 + 128])

    def fire_a2a(i):
        nc.gpsimd.collective_compute(
            "AllToAll", mybir.AluOpType.bypass,
            ins=[cc_in[i]], outs=[cc_out[i]],
            replica_groups=[list(range(N_CORES))],
        )

    for b in range(B):
        for qb in range(NQB):
            q0g = b * T + qb * QB
            njt = (qb + 1) * (QB // 128)
            yps = [
                ypool.tile([HD + 1, QB], F32, tag="yps", name=f"yp{b}_{qb}_{h}")
                for h in range(HPC)
            ]
            for pj in range(njt // 2):
                i0s = []
                sps = [spool.tile([128, 2 * QB], F32, tag="sps", name=f"sp{b}_{qb}_{pj}_{h}")
                       for h in range(HPC)]
                pts = [ptpool.tile([128, 2 * QB], BF16, tag="pt", name=f"pt{b}_{qb}_{pj}_{h}")
                       for h in range(HPC)]
                for jj in range(2):
                    j = 2 * pj + jj
                    j0g = b * T + j * 128
                    i0 = max(0, j * 128 - qb * QB)  # first unmasked query col
                    i0s.append(i0)
                    for h in range(HPC):
                        hs = slice(h * HD, (h + 1) * HD)
                        nc.tensor.matmul(
                            sps[h][:, jj * QB + i0 : (jj + 1) * QB],
                            lhsT=kT[hs, j0g : j0g + 128],
                            rhs=qT[hs, q0g + i0 : q0g + QB], start=True, stop=True,
                        )
                for h in range(HPC):
                    # one exp per j-tile pair; [512+i0s[0] : 512+i0s[1]) holds
                    # exp(garbage) but is never read (PV skips masked cols)
                    nc.scalar.activation(
                        pts[h][:, i0s[0] : 2 * QB], sps[h][:, i0s[0] : 2 * QB],
                        AF.Exp, scale=float(SCALE),
                    )
                    for jj in range(2):
                        j = 2 * pj + jj
                        if j * 128 + 127 > qb * QB:
                            # boundary tile: keep where j <= i on the 128-wide band
                            i0 = i0s[jj]
                            band = slice(jj * QB + i0, jj * QB + i0 + 128)
                            nc.gpsimd.affine_select(
                                pts[h][:, band], pts[h][:, band],
                                pattern=[[1, 128]], base=0, channel_multiplier=-1,
                                compare_op=mybir.AluOpType.is_ge, fill=0.0,
                            )
                for jj in range(2):
                    j = 2 * pj + jj
                    i0 = i0s[jj]
                    for h in range(HPC):
                        nc.tensor.matmul(
                            yps[h][:, i0:QB],
                            lhsT=vsb[:, b * JTN + j, h, 0 : HD + 1],
                            rhs=pts[h][:, jj * QB + i0 : (jj + 1) * QB],
                            start=(j == 0), stop=(j == njt - 1),
                        )
            # softmax normalization: row HD of yp is the denominator.
            # One fast PSUM->SBUF copy releases the yp bank; the recip /
            # broadcast / scale chain then runs off SBUF.
            for h in range(HPC):
                ln = npool.tile([1, QB], F32, tag="ln")
                nc.vector.tensor_copy(ln[:], yps[h][HD : HD + 1, :])
                yraw = npool.tile([HD, QB], F32, tag="yraw")
                nc.vector.tensor_copy(yraw[:], yps[h][0:HD, :])
                rn = npool.tile([1, QB], F32, tag="rn")
                nc.vector.reciprocal_approx_fast(rn[:], ln[:])
                rb = npool.tile([HD, QB], F32, tag="rb")
                nc.gpsimd.partition_broadcast(rb[:], rn[:], channels=HD)
                nc.vector.tensor_mul(yT[h][:, q0g : q0g + QB], yraw[:], rb[:])
            if OVERLAP_A2A:
                stage_qb(b, qb)
                m_last = 16 * b + 4 * qb + 3
                for i in range(4):
                    if m_last == 8 * (CHUNK_B0[i] + CHUNK_L[i]) - 1:
                        fire_a2a(i)

    ph23.close()

    if not OVERLAP_A2A:
        for b in range(B):
            for qb in range(NQB):
                stage_qb(b, qb)
        for i in range(4):
            fire_a2a(i)

    # ---------- phase 3: output projection, pipelined per chunk ----------
    opool = ctx.enter_context(tc.tile_pool(name="ops", bufs=2, space="PSUM"))
    ospool = ctx.enter_context(tc.tile_pool(name="osb", bufs=3))
    yfpool = ctx.enter_context(tc.tile_pool(name="yf", bufs=2))
    for i in range(4):
        L = CHUNK_L[i]
        yfull = yfpool.tile([128, KC, L * 128], BF16, tag="yf", name=f"yf{i}")
        nc.sync.dma_start(yfull[:], cc_out[i].rearrange("r p t -> p r t"))
        for tt in range(L):
            ps0 = opool.tile([128, 512], F32, tag="ops0")
            ps1 = opool.tile([128, 512], F32, tag="ops1")
            for a in range(KC):
                lhsT = yfull[:, a, tt * 128 : (tt + 1) * 128]
                nc.tensor.matmul(ps0[:], lhsT=lhsT, rhs=wp_sb[:, a, 0:512],
                                 start=(a == 0), stop=(a == KC - 1))
                nc.tensor.matmul(ps1[:], lhsT=lhsT, rhs=wp_sb[:, a, 512:C],
                                 start=(a == 0), stop=(a == KC - 1))
            osb = ospool.tile([128, C], F32, tag="osb")
            nc.vector.tensor_add(osb[:, 0:512], ps0[:], bp_sb[:, 0:512])
            nc.vector.tensor_add(osb[:, 512:C], ps1[:], bp_sb[:, 512:C])
            r0 = (CHUNK_B0[i] + tt) * 128
            nc.sync.dma_start(out[r0 : r0 + 128, :], osb[:])


_COMPILED_NC = None


def _get_nc():
    global _COMPILED_NC
    if _COMPILED_NC is None:
        nc = bacc.Bacc("TRN2", target_bir_lowering=False, debug=False,
                       num_devices=N_CORES)
        build_program(nc)
        nc.compile()
        _COMPILED_NC = nc
    return _COMPILED_NC


def kernel(x, W_attn, b_attn, W_proj, b_proj):
    global LAST_RESULTS
    nc = _get_nc()

    bf = ml_dtypes.bfloat16
    xT_np = np.ascontiguousarray(
        np.asarray(x, np.float32).reshape(BT, C).T
    ).astype(bf)
    W_attn = np.asarray(W_attn, np.float32)
    b_attn = np.asarray(b_attn, np.float32)
    W_proj = np.asarray(W_proj, np.float32)
    wp_np = W_proj.astype(bf)
    # b_v folds into b_proj: attention rows sum to 1, so y += b_v exactly
    bp_np = (np.asarray(b_proj, np.float32) + b_attn[2 * C :] @ W_proj).astype(np.float32)

    in_maps = []
    for c in range(N_CORES):
        s = slice(c * FPC, (c + 1) * FPC)
        in_maps.append({
            "xT": xT_np,
            "wq": np.ascontiguousarray(W_attn[:, s]).astype(bf),
            "wk": np.ascontiguousarray(W_attn[:, C:2 * C][:, s]).astype(bf),
            "wv": np.ascontiguousarray(W_attn[:, 2 * C:][:, s]).astype(bf),
            "bqkv": np.ascontiguousarray(
                np.stack([b_attn[s], b_attn[C:2 * C][s], b_attn[2 * C:][s]])
            ).astype(np.float32),
            "wp": wp_np,
            "bp": bp_np,
        })

    res = run_bass_kernel_spmd(nc, in_maps, core_ids=list(range(N_CORES)))
    LAST_RESULTS = res
    # round-robin gather: core c's row-block j is global 128-row block 8j+c
    arr = np.stack([res.results[c]["out"] for c in range(N_CORES)], axis=0)
    full = arr.reshape(N_CORES, ROWS // 128, 128, C).transpose(1, 0, 2, 3)
    return np.ascontiguousarray(full).reshape(B, T, C)


# revision 12
# speedup vs baseline: 1.0443x; 1.0443x over previous
"""Causal self-attention (GPT-style block) on 8 Trainium2 NeuronCores.

Problem: x[4,2048,1024] -> qkv = x@W_attn+b ; 16-head causal attention
(head_dim 64) ; out = y@W_proj+b_proj.

Sharding: tensor-parallel over heads. Core c owns heads {2c, 2c+1}:
  - computes q^T/k^T (feature-major) and v (key-major, natural layout)
    for its heads over the full batch via matmuls against a host-
    pretransposed x^T (bf16). v-natural comes from using the x^T tile as
    the stationary operand (out partitions = tokens), so no DMA
    transposes / DRAM round trip are needed.
  - runs causal attention for its 8 (batch, head) pairs entirely in SBUF
    (S^T layout: scores tile [128 j, 512 i]; j-tiles processed in pairs
    sharing one [128, 1024] PSUM tile so exp runs as one ScalarE
    activation per pair; causal mask via affine_select on the diagonal
    band only - fully masked leading columns are simply never read,
    because the PV matmul accumulates into yps[:, i0:] per tile; PV
    appends a ones-column to v producing y_raw^T and the softmax
    denominator in one PSUM tile),
  - per batch, a striped AllToAll (launched as soon as that batch's
    attention finishes, overlapping the next batch's compute) swaps the
    head dim for the row dim; the output projection for the core's
    4x256 rows runs after attention, pipelined with the last collective.

Bias handling: b_k drops out of softmax exactly (constant per query row);
b_v is folded into b_proj on the host (attention rows sum to 1); only
b_q is applied on device.

Numerics: bf16 operands with fp32 PSUM accumulation everywhere; softmax
skips the max-subtraction (scores are O(1) by construction; exp stays
finite) which matches the reference to ~1e-5 in fp32.
"""

import numpy as np
import ml_dtypes
from contextlib import ExitStack

import concourse.bass as bass
import concourse.tile as tile
from concourse import bacc, mybir
from concourse.bass_utils import run_bass_kernel_spmd

F32 = mybir.dt.float32
BF16 = mybir.dt.bfloat16
AF = mybir.ActivationFunctionType

N_CORES = 8
B, T, C, H = 4, 2048, 1024, 16
HD = C // H            # 64 head dim
HPC = H // N_CORES     # 2 heads per core
FPC = HPC * HD         # 128 features per core
BT = B * T             # 8192 rows
TCHUNK = 512           # t chunk in qkv phase
NT_CHUNKS = BT // TCHUNK
QB = 512               # query block
NQB = T // QB          # 4 per batch
JTN = T // 128         # 16 j-tiles per batch
ROWS = BT // N_CORES   # 1024 rows per core after AllToAll
KC = C // 128          # 8 contraction tiles over C
VW = 80                # [V | 1 | pad] row unit in vsb
STRIPE = ROWS // B     # 256 rows per (core, batch): striped AllToAll
SCALE = 1.0 / np.sqrt(HD)

# If the overlapped (per-batch, concurrent-with-compute) AllToAll turns
# out to corrupt data on HW, set False to emit all collectives after the
# attention loop (still chunked and pipelined with the out projection).
OVERLAP_A2A = True

# AllToAll chunking: chunk i ships CHUNK_L[i] 128-row blocks per core.
# Chunks 0-2 are batches 0-2 (dest r takes that batch's contiguous rows
# 2r,2r+1 -> one contiguous stage DMA per dest+head, clustered per batch:
# scattered small staging DMAs during attention deepen HW power
# throttling). Batch 3 splits into two 256KB chunks (dest r takes block
# 48+r / 56+r) so the collective gating the tail is small and the first
# half fires two qb-blocks before attention ends.
CHUNK_L = (2, 2, 2, 1, 1)
CHUNK_B0 = (0, 2, 4, 6, 7)  # cumulative blocks-per-dest before each chunk

# global 128-row blocks owned by core c, in device row order
def blocks_of_core(c):
    return [2 * c, 2 * c + 1, 16 + 2 * c, 16 + 2 * c + 1,
            32 + 2 * c, 32 + 2 * c + 1, 48 + c, 56 + c]

LAST_RESULTS = None    # test.py reads exec_time_ns off this


def build_program(nc):
    xT = nc.dram_tensor("xT", [C, BT], BF16, kind="ExternalInput").ap()
    wq = nc.dram_tensor("wq", [C, FPC], BF16, kind="ExternalInput").ap()
    wk = nc.dram_tensor("wk", [C, FPC], BF16, kind="ExternalInput").ap()
    wv = nc.dram_tensor("wv", [C, FPC], BF16, kind="ExternalInput").ap()
    bqkv = nc.dram_tensor("bqkv", [3, FPC], F32, kind="ExternalInput").ap()
    wp = nc.dram_tensor("wp", [C, C], BF16, kind="ExternalInput").ap()
    bp = nc.dram_tensor("bp", [C], F32, kind="ExternalInput").ap()
    out = nc.dram_tensor("out", [ROWS, C], F32, kind="ExternalOutput").ap()
    # round-robin 128-row blocks: global block m (of 64) -> core m%8, so a
    # collective chunk covering any 8*L consecutive blocks addresses all 8
    # destinations. Chunks fire after b1, b2, (b3,qb1), (b3,qb3) with
    # L = 4, 2, 1, 1 blocks per destination.
    cc_in = [
        nc.dram_tensor(f"cc_in{i}", [N_CORES, FPC, L * 128], BF16, kind="Internal").ap()
        for i, L in enumerate(CHUNK_L)
    ]
    cc_out = [
        nc.dram_tensor(f"cc_out{i}", [N_CORES, FPC, L * 128], BF16, kind="Internal").ap()
        for i, L in enumerate(CHUNK_L)
    ]

    with tile.TileContext(nc) as tc:
        with ExitStack() as ctx:
            emit(ctx, tc, xT, wq, wk, wv, bqkv, wp, bp, out, cc_in, cc_out)
    return nc


def emit(ctx, tc, xT, wq, wk, wv, bqkv, wp, bp, out, cc_in, cc_out):
    nc = tc.nc
    res = ctx.enter_context(tc.tile_pool(name="resident", bufs=1))

    # ---------- resident SBUF ----------
    qT = res.tile([128, BT], BF16)
    kT = res.tile([128, BT], BF16)
    vsb = res.tile([128, B * JTN, HPC, VW], BF16)     # [V | 1 | pad] per j-tile/head
    wq_sb = res.tile([128, KC, FPC], BF16)
    wk_sb = res.tile([128, KC, FPC], BF16)
    wv_sb = res.tile([128, KC, FPC], BF16)
    b_sb = res.tile([128, 3], F32)
    wp_sb = res.tile([128, KC, C], BF16)
    bp_sb = res.tile([128, C], F32)
    yT0 = res.tile([64, BT], BF16)
    yT1 = res.tile([64, BT], BF16)

    # ---------- constant/weight loads (wp/bp deferred to phase 3) ----------
    nc.sync.dma_start(wq_sb[:], wq.rearrange("(a p) m -> p a m", p=128))
    nc.sync.dma_start(wk_sb[:], wk.rearrange("(a p) m -> p a m", p=128))
    nc.sync.dma_start(wv_sb[:], wv.rearrange("(a p) m -> p a m", p=128))
    nc.sync.dma_start(b_sb[:], bqkv.rearrange("b p -> p b"))
    nc.vector.memset(vsb[:, :, :, HD : HD + 1], 1.0)

    # ---------- phase 1: q^T, k^T (feature-major) and v (natural) ----------
    ph12 = ExitStack()
    xpool = ph12.enter_context(tc.tile_pool(name="xt", bufs=3))
    qkvps = ph12.enter_context(tc.tile_pool(name="qkvps", bufs=2, space="PSUM"))
    vps = ph12.enter_context(tc.tile_pool(name="vps", bufs=2, space="PSUM"))
    xT_t = xT.rearrange("(a p) t -> p a t", p=128)
    NSUB = TCHUNK // 128
    for tci in range(NT_CHUNKS):
        t0 = tci * TCHUNK
        xt = xpool.tile([128, KC, TCHUNK], BF16, tag="xt")
        # split the 1 MiB chunk load across 4 DMA queues
        for spl in range(4):
            nc.sync.dma_start(
                xt[:, 2 * spl : 2 * spl + 2, :],
                xT_t[:, 2 * spl : 2 * spl + 2, t0 : t0 + TCHUNK],
            )
        for w_sb, bi, dst in ((wq_sb, 0, qT), (wk_sb, 1, kT)):
            ps = qkvps.tile([128, TCHUNK], F32, tag="qkvps")
            for a in range(KC):
                nc.tensor.matmul(
                    ps[:], lhsT=w_sb[:, a, :], rhs=xt[:, a, :],
                    start=(a == 0), stop=(a == KC - 1),
                )
            # evictions on DVE: keeps ScalarE exp-only (no ACT table switches)
            if bi == 0:
                nc.vector.tensor_scalar_add(
                    dst[:, t0 : t0 + TCHUNK], ps[:], b_sb[:, bi : bi + 1]
                )
            else:
                # b_k shifts every score in a query row equally -> softmax
                # invariant; drop it.
                nc.vector.tensor_copy(dst[:, t0 : t0 + TCHUNK], ps[:])
        # v in natural layout: out partitions = tokens (keys), free = feature.
        # lhsT = x^T tile (stationary), rhs = W_v tile (moving).
        vp = vps.tile([128, NSUB, HPC, HD], F32, tag="vps")
        for tt in range(NSUB):
            for a in range(KC):
                nc.tensor.matmul(
                    vp[:, tt, :, :],
                    lhsT=xt[:, a, tt * 128 : (tt + 1) * 128],
                    rhs=wv_sb[:, a, :],
                    start=(a == 0), stop=(a == KC - 1),
                )
        # b_v is folded into b_proj on the host (attention rows sum to 1)
        g0 = NSUB * tci
        nc.vector.tensor_copy(vsb[:, g0 : g0 + NSUB, :, 0:HD], vp[:])

    ph12.close()  # release phase-1 PSUM/xt pools before the attention pools open

    nc.sync.dma_start(wp_sb[:], wp.rearrange("(a p) e -> p a e", p=128))
    bp_bcast = bass.AP(tensor=bp.tensor, offset=bp.offset, ap=[[0, 128], [1, C]])
    nc.sync.dma_start(bp_sb[:], bp_bcast)

    # ---------- phase 2: causal attention, S^T layout, heads interleaved ----------
    # Interleaving the two heads keeps consecutive PE matmuls independent
    # (different array row groups for S, different PSUM banks throughout),
    # so LDWEIGHTS/fill/drain overlap instead of serializing.
    ph23 = ExitStack()
    spool = ph23.enter_context(tc.tile_pool(name="sps", bufs=3, space="PSUM"))
    ypool = ph23.enter_context(tc.tile_pool(name="yps", bufs=2, space="PSUM"))
    ptpool = ph23.enter_context(tc.tile_pool(name="pt", bufs=3))
    npool = ph23.enter_context(tc.tile_pool(name="norm", bufs=3))
    yT = (yT0, yT1)

    def stage_batch(b):
        # chunk b: dest r gets the batch's contiguous rows r*256..+256
        for r in range(N_CORES):
            c0 = b * T + r * 2 * 128
            nc.sync.dma_start(cc_in[b][r, 0:HD, :], yT0[:, c0 : c0 + 256])
            nc.sync.dma_start(cc_in[b][r, HD:FPC, :], yT1[:, c0 : c0 + 256])

    def stage_b3_qb(qb):
        # chunk 3+qb//2: dest r gets batch-3 block 4*(qb%2)..: one block each
        i = 3 + qb // 2
        for u in range(4):
            r = 4 * (qb % 2) + u
            c0 = 3 * T + (4 * qb + u) * 128
            nc.sync.dma_start(cc_in[i][r, 0:HD, :], yT0[:, c0 : c0 + 128])
            nc.sync.dma_start(cc_in[i][r, HD:FPC, :], yT1[:, c0 : c0 + 128])

    def fire_a2a(i):
        nc.gpsimd.collective_compute(
            "AllToAll", mybir.AluOpType.bypass,
            ins=[cc_in[i]], outs=[cc_out[i]],
            replica_groups=[list(range(N_CORES))],
        )

    for b in range(B):
        for qb in range(NQB):
            q0g = b * T + qb * QB
            njt = (qb + 1) * (QB // 128)
            yps = [
                ypool.tile([HD + 1, QB], F32, tag="yps", name=f"yp{b}_{qb}_{h}")
                for h in range(HPC)
            ]
            for pj in range(njt // 2):
                i0s = []
                sps = [spool.tile([128, 2 * QB], F32, tag="sps", name=f"sp{b}_{qb}_{pj}_{h}")
                       for h in range(HPC)]
                pts = [ptpool.tile([128, 2 * QB], BF16, tag="pt", name=f"pt{b}_{qb}_{pj}_{h}")
                       for h in range(HPC)]
                for jj in range(2):
                    j = 2 * pj + jj
                    j0g = b * T + j * 128
                    i0 = max(0, j * 128 - qb * QB)  # first unmasked query col
                    i0s.append(i0)
                    for h in range(HPC):
                        hs = slice(h * HD, (h + 1) * HD)
                        nc.tensor.matmul(
                            sps[h][:, jj * QB + i0 : (jj + 1) * QB],
                            lhsT=kT[hs, j0g : j0g + 128],
                            rhs=qT[hs, q0g + i0 : q0g + QB], start=True, stop=True,
                        )
                for h in range(HPC):
                    # one exp per j-tile pair; [512+i0s[0] : 512+i0s[1]) holds
                    # exp(garbage) but is never read (PV skips masked cols)
                    nc.scalar.activation(
                        pts[h][:, i0s[0] : 2 * QB], sps[h][:, i0s[0] : 2 * QB],
                        AF.Exp, scale=float(SCALE),
                    )
                    for jj in range(2):
                        j = 2 * pj + jj
                        if j * 128 + 127 > qb * QB:
                            # boundary tile: keep where j <= i on the 128-wide band
                            i0 = i0s[jj]
                            band = slice(jj * QB + i0, jj * QB + i0 + 128)
                            nc.gpsimd.affine_select(
                                pts[h][:, band], pts[h][:, band],
                                pattern=[[1, 128]], base=0, channel_multiplier=-1,
                                compare_op=mybir.AluOpType.is_ge, fill=0.0,
                            )
                for jj in range(2):
                    j = 2 * pj + jj
                    i0 = i0s[jj]
                    for h in range(HPC):
                        nc.tensor.matmul(
                            yps[h][:, i0:QB],
                            lhsT=vsb[:, b * JTN + j, h, 0 : HD + 1],
                            rhs=pts[h][:, jj * QB + i0 : (jj + 1) * QB],
                            start=(j == 0), stop=(j == njt - 1),
                        )
            # softmax normalization: row HD of yp is the denominator.
            # One fast PSUM->SBUF copy releases the yp bank; the recip /
            # broadcast / scale chain then runs off SBUF.
            for h in range(HPC):
                ln = npool.tile([1, QB], F32, tag="ln")
                nc.vector.tensor_copy(ln[:], yps[h][HD : HD + 1, :])
                yraw = npool.tile([HD, QB], F32, tag="yraw")
                nc.vector.tensor_copy(yraw[:], yps[h][0:HD, :])
                rn = npool.tile([1, QB], F32, tag="rn")
                nc.vector.reciprocal_approx_fast(rn[:], ln[:])
                rb = npool.tile([HD, QB], F32, tag="rb")
                nc.gpsimd.partition_broadcast(rb[:], rn[:], channels=HD)
                nc.vector.tensor_mul(yT[h][:, q0g : q0g + QB], yraw[:], rb[:])
            if OVERLAP_A2A:
                if b < 3 and qb == 3:
                    stage_batch(b)
                    fire_a2a(b)
                elif b == 3:
                    stage_b3_qb(qb)
                    if qb % 2 == 1:
                        fire_a2a(3 + qb // 2)

    ph23.close()

    if not OVERLAP_A2A:
        for b in range(3):
            stage_batch(b)
        for qb in range(NQB):
            stage_b3_qb(qb)
        for i in range(5):
            fire_a2a(i)

    # ---------- phase 3: output projection, pipelined per chunk ----------
    opool = ctx.enter_context(tc.tile_pool(name="ops", bufs=2, space="PSUM"))
    ospool = ctx.enter_context(tc.tile_pool(name="osb", bufs=3))
    yfpool = ctx.enter_context(tc.tile_pool(name="yf", bufs=2))
    for i in range(5):
        L = CHUNK_L[i]
        yfull = yfpool.tile([128, KC, L * 128], BF16, tag="yf", name=f"yf{i}")
        nc.sync.dma_start(yfull[:], cc_out[i].rearrange("r p t -> p r t"))
        for tt in range(L):
            ps0 = opool.tile([128, 512], F32, tag="ops0")
            ps1 = opool.tile([128, 512], F32, tag="ops1")
            for a in range(KC):
                lhsT = yfull[:, a, tt * 128 : (tt + 1) * 128]
                nc.tensor.matmul(ps0[:], lhsT=lhsT, rhs=wp_sb[:, a, 0:512],
                                 start=(a == 0), stop=(a == KC - 1))
                nc.tensor.matmul(ps1[:], lhsT=lhsT, rhs=wp_sb[:, a, 512:C],
                                 start=(a == 0), stop=(a == KC - 1))
            osb = ospool.tile([128, C], F32, tag="osb")
            nc.vector.tensor_add(osb[:, 0:512], ps0[:], bp_sb[:, 0:512])
            nc.vector.tensor_add(osb[:, 512:C], ps1[:], bp_sb[:, 512:C])
            r0 = (CHUNK_B0[i] + tt) * 128
            nc.sync.dma_start(out[r0 : r0 + 128, :], osb[:])


_COMPILED_NC = None


def _get_nc():
    global _COMPILED_NC
    if _COMPILED_NC is None:
        nc = bacc.Bacc("TRN2", target_bir_lowering=False, debug=False,
                       num_devices=N_CORES)
        build_program(nc)
        nc.compile()
        _COMPILED_NC = nc
    return _COMPILED_NC


def kernel(x, W_attn, b_attn, W_proj, b_proj):
    global LAST_RESULTS
    nc = _get_nc()

    bf = ml_dtypes.bfloat16
    xT_np = np.ascontiguousarray(
        np.asarray(x, np.float32).reshape(BT, C).T
    ).astype(bf)
    W_attn = np.asarray(W_attn, np.float32)
    b_attn = np.asarray(b_attn, np.float32)
    W_proj = np.asarray(W_proj, np.float32)
    wp_np = W_proj.astype(bf)
    # b_v folds into b_proj: attention rows sum to 1, so y += b_v exactly
    bp_np = (np.asarray(b_proj, np.float32) + b_attn[2 * C :] @ W_proj).astype(np.float32)

    in_maps = []
    for c in range(N_CORES):
        s = slice(c * FPC, (c + 1) * FPC)
        in_maps.append({
            "xT": xT_np,
            "wq": np.ascontiguousarray(W_attn[:, s]).astype(bf),
            "wk": np.ascontiguousarray(W_attn[:, C:2 * C][:, s]).astype(bf),
            "wv": np.ascontiguousarray(W_attn[:, 2 * C:][:, s]).astype(bf),
            "bqkv": np.ascontiguousarray(
                np.stack([b_attn[s], b_attn[C:2 * C][s], b_attn[2 * C:][s]])
            ).astype(np.float32),
            "wp": wp_np,
            "bp": bp_np,
        })

    res = run_bass_kernel_spmd(nc, in_maps, core_ids=list(range(N_CORES)))
    LAST_RESULTS = res
    # gather: core c's j-th 128-row block is global block blocks_of_core(c)[j]
    arr = np.stack([res.results[c]["out"] for c in range(N_CORES)], axis=0)
    arr = arr.reshape(N_CORES, ROWS // 128, 128, C)
    full = np.empty((BT // 128, 128, C), np.float32)
    for c in range(N_CORES):
        full[blocks_of_core(c)] = arr[c]
    return full.reshape(B, T, C)
